# revision 37
# baseline (speedup 1.0000x reference)
"""CrossAttention Trainium2 kernel (8 NeuronCores, SPMD), bf16 compute.

Sharding: data-parallel over batch B=2, tensor-parallel over the 16 heads in
4 groups of 4 heads -> 8 cores, one (batch, head-group) pair each. Each core
computes its 4 heads' Q/K/V projections, masked softmax cross-attention, and
its partial output projection y_g = softmax(q k^T * scale) v @ Wo[:, g].T.
The host sums the 4 partial outputs per batch (the Wo row-split all-reduce,
done at unshard time) and adds the v-bias term Wo @ b_v, which is constant
across rows and factors out of the attention (softmax rows sum to 1).
The k-bias is dropped entirely: (k+b)^T q adds a per-query constant to the
logits, which softmax over keys is invariant to.

Numerics: inputs are cast to bf16 on the host; every matmul runs bf16 x bf16
with fp32 PSUM accumulation; softmax statistics (denominator, reciprocal,
normalization) stay fp32. Output partials are written bf16 and summed fp32
on the host. End-to-end relative error ~3e-3.

Layout: the PE contracts over the partition dim, so activations and weights
arrive contraction-major (pre-transposed on host); every device DMA is a
plain contiguous row load; no transposes on the device.

Attention is computed scores-transposed: ST[m, n] per head so PV contracts
over m directly; the two heads of a pair sit at partitions 0:64 / 64:128 so
their score matmuls run concurrently as PE row-tiles. The softmax
denominator comes free from a ones-column appended to v. exp() is
unnormalized (|s*scale| < ~4); mask zeros are applied multiplicatively
after exp with one wide (free=2048) DVE multiply per (pair, m-tile).

Stages (PE-dense, ACT overlapped):
  Q proj (chunk-major, DMA-paced) ->
  stage 1: scores+exp+mask heads 0,1 with K and V projections on PE slack
  stage 2: PV heads 0,1 interleaved with scores+exp+mask heads 2,3
  stage 3: PV head 2 (normalize heads 0,1 in its shadow), PV head 3
           (normalize head 2), normalize head 3, output projection.
Normalization: the denominator row is broadcast to 64 partitions with a
K=1 ones matmul on the PE (rhs read from the evicted ot_sb row at
partition 64), then DVE reciprocal + scale; odd heads shift into the upper
partition half via SBUF-SBUF DMA.
"""

import os

import numpy as np
import ml_dtypes

import concourse.bass as bass
import concourse.bacc as bacc
import concourse.mybir as mybir
import concourse.tile as tile
from concourse.bass_utils import run_bass_kernel_spmd

DIM = 1024
HEAD_DIM = 64
NUM_HEADS = 16
SCALE = HEAD_DIM**-0.5
B, N, M = 2, 1024, 2048
HPC = 4  # heads per core
E = HPC * HEAD_DIM  # 256: per-core projection width
P = 128
F32 = mybir.dt.float32
BF16 = mybir.dt.bfloat16
CT = DIM // P  # 8 contraction tiles
MT = M // P  # 16 m tiles


def build_program():
    nc = bacc.Bacc("TRN2", target_bir_lowering=False, debug=False, num_devices=8)

    # activation/weight shards arrive partition-major ([p, j, ...] with the
    # SBUF partition index outermost, pre-packed on the host) so every load
    # is one DMA with multi-KB contiguous runs per partition
    xT_d = nc.dram_tensor("xT", [P, CT, N], BF16, kind="ExternalInput").ap()
    ctxT_d = nc.dram_tensor("ctxT", [P, CT, M], BF16, kind="ExternalInput").ap()
    maskt_d = nc.dram_tensor("maskt", [P, MT, N], BF16, kind="ExternalInput").ap()
    wqT_d = nc.dram_tensor("wqT", [P, CT, E], BF16, kind="ExternalInput").ap()
    wkT_d = nc.dram_tensor("wkT", [P, CT, E], BF16, kind="ExternalInput").ap()
    wvT_d = nc.dram_tensor("wvT", [P, CT, E], BF16, kind="ExternalInput").ap()
    woT_d = nc.dram_tensor("woT", [P, E // P, DIM], BF16, kind="ExternalInput").ap()
    y_d = nc.dram_tensor("y", [N, DIM], BF16, kind="ExternalOutput").ap()

    kdbg = bool(os.environ.get("KDBG"))
    if kdbg:
        otdump_d = nc.dram_tensor(
            "otdump", [HEAD_DIM + 1, HPC, N], F32, kind="ExternalOutput"
        ).ap()

    Exp = mybir.ActivationFunctionType.Exp

    from contextlib import ExitStack

    with tile.TileContext(nc) as tc, ExitStack() as ctx:
        persist = ctx.enter_context(tc.tile_pool(name="persist", bufs=1))
        qT = persist.tile([P, E // P, N], BF16)
        kT = persist.tile([P, E // P, M], BF16)
        # v columns padded to 128 so the PV stationary is a full-width
        # weight load (enables Fast Weight Load; col 64 = ones for the
        # softmax denominator, cols 65:128 = don't-care)
        vaug = persist.tile([P, MT, HPC, P], BF16)
        woT = persist.tile([P, E // P, DIM], BF16)
        otn2 = persist.tile([P, E // P, N], BF16)
        ones_sb = persist.tile([P, HEAD_DIM], BF16)
        # rows 0:64 unnormalized attention out, row 64 denominator (bf16:
        # the ~0.4% rounding is well within the error budget and halves
        # the SBUF footprint; softmax statistics stay fp32 downstream)
        ot_sb = persist.tile([HEAD_DIM + 1, HPC, N], BF16)

        # ones column: fill everything; v evictions overwrite cols 0:64
        nc.vector.memset(vaug, 1.0)
        nc.vector.memset(ones_sb, 1.0)

        bwork = ctx.enter_context(tc.tile_pool(name="bwork", bufs=2))
        maskp = ctx.enter_context(tc.tile_pool(name="maskp", bufs=2))
        rbp = ctx.enter_context(tc.tile_pool(name="rbp", bufs=1))

        def emit_scores(spool, sbufs, hp, mt, exmst, mk):
            """scores -> exp for head pair hp at m-tile mt (per n-chunk PSUM
            tiles, double-buffered), then one wide masked multiply."""
            ex = bwork.tile([P, 2, N], BF16, tag="ex", name="ex")
            for chn in range(N // 512):
                st = spool.tile(
                    [P, 2, 512], F32, tag="st", name="st", bufs=sbufs
                )
                for hl in range(2):
                    erow = slice(hl * HEAD_DIM, (hl + 1) * HEAD_DIM)
                    nc.tensor.matmul(
                        st[:, hl, :],
                        lhsT=kT[erow, hp, mt * P : (mt + 1) * P],
                        rhs=qT[erow, hp, chn * 512 : (chn + 1) * 512],
                        start=True,
                        stop=True,
                    )
                nc.scalar.activation(
                    ex[:, :, chn * 512 : (chn + 1) * 512], st, Exp,
                    scale=float(SCALE),
                )
            mkc = bass.AP(mk.tensor, mk.offset, [mk.ap[0], [0, 2], mk.ap[1]])
            nc.vector.tensor_mul(exmst[:, mt, :, :], ex, mkc)

        def emit_pv(ot_ps, hp, mt, exmst):
            for hl in range(2):
                h = hp * 2 + hl
                for chn in range(N // 512):
                    nc.tensor.matmul(
                        ot_ps[hl * 2 + chn],
                        lhsT=vaug[:, mt, h, :],
                        rhs=exmst[:, mt, hl, chn * 512 : (chn + 1) * 512],
                        start=(mt == 0),
                        stop=(mt == MT - 1),
                    )

        def emit_pv1(ot_ps, h, mt, exmst):
            """PV for a single head h; ot_ps = [chn0, chn1] psum tiles."""
            hl = h % 2
            for chn in range(N // 512):
                nc.tensor.matmul(
                    ot_ps[chn],
                    lhsT=vaug[:, mt, h, :],
                    rhs=exmst[:, mt, hl, chn * 512 : (chn + 1) * 512],
                    start=(mt == 0),
                    stop=(mt == MT - 1),
                )

        def evict_head(ot_ps, h):
            """copy head h's two PV accumulators to ot_sb: denominator row
            first on the (idle) ACT engine so the normalize broadcast can
            start before the value rows land via DVE."""
            dn = slice(HEAD_DIM, HEAD_DIM + 1)
            for chn in range(2):
                nc.scalar.copy(
                    ot_sb[dn, h, chn * 512 : (chn + 1) * 512],
                    ot_ps[chn][dn, :],
                )
            for chn in range(2):
                nc.vector.tensor_copy(
                    ot_sb[:HEAD_DIM, h, chn * 512 : (chn + 1) * 512],
                    ot_ps[chn][:HEAD_DIM, :],
                )

        def normalize_head(h, rbq):
            """softmax-normalize head h from ot_sb into its otn2 half. The
            denominator row (partition 64) is broadcast to partitions 0:64
            with a K=1 bf16 ones matmul on the PE: rb_ps[d,n] = den[n]."""
            hp, hl = divmod(h, 2)
            dn = slice(HEAD_DIM, HEAD_DIM + 1)
            rb_ps = rbq.tile(
                [HEAD_DIM, N], F32, tag="rbps", name="rbps", bufs=1
            )
            for chn in range(2):
                nc.tensor.matmul(
                    rb_ps[:, chn * 512 : (chn + 1) * 512],
                    lhsT=ones_sb[HEAD_DIM : HEAD_DIM + 1, :],
                    rhs=ot_sb[dn, h, chn * 512 : (chn + 1) * 512],
                    start=True,
                    stop=True,
                )
            rb = rbp.tile([HEAD_DIM, N], F32, tag="rb", name="rb")
            nc.vector.reciprocal_approx_fast(out=rb, in_=rb_ps)
            if hl == 0:
                nc.vector.tensor_mul(
                    otn2[:HEAD_DIM, hp, :], ot_sb[:HEAD_DIM, h, :], rb
                )
            else:
                tmp = rbp.tile([HEAD_DIM, N], BF16, tag="tmp", name="tmp")
                nc.vector.tensor_mul(tmp, ot_sb[:HEAD_DIM, h, :], rb)
                # partition shift 0:64 -> 64:128 via SBUF-SBUF DMA
                nc.sync.dma_start(out=otn2[HEAD_DIM:P, hp, :], in_=tmp)

        def load_mask_group(g):
            """one 1MB DMA covering four m-tiles of the mask (big transfers
            spread over all 16 SDMA engines at ~3x the small-DMA rate)."""
            mkg = maskp.tile([P, 4, N], BF16, tag="mkg", name="mkg")
            nc.gpsimd.dma_start(out=mkg, in_=maskt_d[:, 4 * g : 4 * (g + 1), :])
            return mkg

        with tc.tile_pool(name="exmp", bufs=1) as exmp:
            # masked exp(scores) parked per m-tile; one buffer reused across
            # head pairs (WAR: stage-2 rewrites a tile only after its PV read)
            exmst = exmp.tile([P, MT, 2, N], BF16)

            with tc.tile_pool(name="wctx", bufs=1) as wctx_pool:
                wkT = wctx_pool.tile([P, CT, E], BF16)
                wvT = wctx_pool.tile([P, CT, E], BF16)
                ctxT = wctx_pool.tile([P, CT, M], BF16)

                with tc.tile_pool(name="qx", bufs=1) as qx_pool:
                    wqT = qx_pool.tile([P, CT, E], BF16)
                    xT = qx_pool.tile([P, CT, N], BF16)
                    # few, large DMAs: a single dma_start is split across all
                    # 16 SDMA engines and the partition-major host packing
                    # gives multi-KB contiguous runs (near-peak HBM rate).
                    # Dependency-first order; sync+scalar are the two HWDGE
                    # rings, gpsimd is the software ring.
                    nc.sync.dma_start(out=wqT, in_=wqT_d)
                    nc.sync.dma_start(out=xT[:, :, 0:512], in_=xT_d[:, :, 0:512])
                    nc.sync.dma_start(out=xT[:, :, 512:N], in_=xT_d[:, :, 512:N])
                    nc.scalar.dma_start(out=wkT, in_=wkT_d)
                    nc.scalar.dma_start(
                        out=ctxT[:, :, 0:512], in_=ctxT_d[:, :, 0:512]
                    )
                    nc.scalar.dma_start(
                        out=ctxT[:, :, 512:1024], in_=ctxT_d[:, :, 512:1024]
                    )
                    nc.sync.dma_start(
                        out=ctxT[:, :, 1024:M], in_=ctxT_d[:, :, 1024:M]
                    )
                    nc.gpsimd.dma_start(out=wvT, in_=wvT_d)
                    nc.gpsimd.dma_start(out=woT, in_=woT_d)

                    # Q projection, chunk-major so the first psum tile only
                    # needs wq + the first x half-tiles
                    with tc.tile_pool(name="ppsA", bufs=3, space="PSUM") as ppsA:
                        for chn in range(N // 512):
                            for et in range(E // P):
                                pq = ppsA.tile([P, 512], F32, tag="pq")
                                for j in range(CT):
                                    nc.tensor.matmul(
                                        pq,
                                        lhsT=wqT[:, j, et * P : (et + 1) * P],
                                        rhs=xT[:, j, chn * 512 : (chn + 1) * 512],
                                        start=(j == 0),
                                        stop=(j == CT - 1),
                                    )
                                nc.vector.tensor_copy(
                                    qT[:, et, chn * 512 : (chn + 1) * 512], pq
                                )

                def emit_kproj(kps, et, chm):
                    pk = kps.tile([P, 512], F32, tag="pk", name="pk")
                    for j in range(CT):
                        nc.tensor.matmul(
                            pk,
                            lhsT=wkT[:, j, et * P : (et + 1) * P],
                            rhs=ctxT[:, j, chm * 512 : (chm + 1) * 512],
                            start=(j == 0),
                            stop=(j == CT - 1),
                        )
                    nc.vector.tensor_copy(
                        kT[:, et, chm * 512 : (chm + 1) * 512], pk
                    )

                # K projection for the first head pair's first chunk must
                # precede stage 1; the rest is folded into stage 1's PE slack.
                # stage 1: scores(heads 0,1) [ACT-bound] + V and K
                # projections interleaved on the otherwise idle PE.
                with (
                    tc.tile_pool(name="sps1", bufs=1, space="PSUM") as sps1,
                    tc.tile_pool(name="vps", bufs=2, space="PSUM") as vps,
                    tc.tile_pool(name="kps", bufs=2, space="PSUM") as kps,
                ):
                    # kproj pacing: late enough that the tensor queue never
                    # blocks on not-yet-arrived ctx columns, early enough
                    # that kT stays ahead of the scores that consume it
                    # (et0 chunk c feeds scores mts 4c..4c+3; et1 feeds
                    # stage 2).
                    kproj_at = {4: (0, 1), 6: (0, 2), 8: (0, 3),
                                9: (1, 0), 11: (1, 1), 13: (1, 2), 15: (1, 3)}
                    emit_kproj(kps, 0, 0)
                    mkg = None
                    for mt in range(MT):
                        if et_chm := kproj_at.get(mt):
                            emit_kproj(kps, *et_chm)
                        if mt % 4 == 0:
                            mkg = load_mask_group(mt // 4)
                        emit_scores(sps1, 2, 0, mt, exmst, mkg[:, mt % 4, :])
                        pv = vps.tile([P, HPC, HEAD_DIM], F32, tag="pv")
                        for j in range(CT):
                            nc.tensor.matmul(
                                pv,
                                lhsT=ctxT[:, j, mt * P : (mt + 1) * P],
                                rhs=wvT[:, j, :],
                                start=(j == 0),
                                stop=(j == CT - 1),
                            )
                        # single fused eviction of all 4 heads' v columns
                        nc.vector.tensor_copy(
                            vaug[:, mt, :, :HEAD_DIM], pv
                        )

            # stage 2: PV(heads 0,1) interleaved with scores(heads 2,3)
            with tc.tile_pool(name="ops0", bufs=1, space="PSUM") as ops0:
                ot_ps0 = [
                    ops0.tile([P, 512], F32, tag=f"o{i}", name=f"o{i}")
                    for i in range(4)
                ]
                with tc.tile_pool(name="sps2", bufs=1, space="PSUM") as sps2:
                    mkg = None
                    for mt in range(MT):
                        if mt % 4 == 0:
                            mkg = load_mask_group(mt // 4)
                        emit_pv(ot_ps0, 0, mt, exmst)
                        emit_scores(sps2, 2, 1, mt, exmst, mkg[:, mt % 4, :])
                evict_head(ot_ps0[0:2], 0)
                evict_head(ot_ps0[2:4], 1)

            # stage 3: PV head 3 first (normalize heads 0,1 in its shadow),
            # then PV head 2; head 3's partition-shift DMA overlaps head 2's
            # PV so the last head (2, even -> no shift) has the shortest
            # possible normalize chain before the output projection.
            with (
                tc.tile_pool(name="ops1", bufs=1, space="PSUM") as ops1,
                tc.tile_pool(name="rbq", bufs=1, space="PSUM") as rbq,
            ):
                ot_ps3 = [
                    ops1.tile([P, 512], F32, tag=f"q{i}", name=f"q{i}")
                    for i in range(2)
                ]
                for mt in range(MT):
                    emit_pv1(ot_ps3, 3, mt, exmst)
                    if mt == 2:
                        normalize_head(0, rbq)
                    if mt == 8:
                        normalize_head(1, rbq)
                evict_head(ot_ps3, 3)
                ot_ps2 = [
                    ops1.tile([P, 512], F32, tag=f"p{i}", name=f"p{i}")
                    for i in range(2)
                ]
                for mt in range(MT):
                    emit_pv1(ot_ps2, 2, mt, exmst)
                    if mt == 2:
                        normalize_head(3, rbq)
                evict_head(ot_ps2, 2)
                normalize_head(2, rbq)

            if kdbg:
                nc.sync.dma_start(out=otdump_d, in_=ot_sb)

        # ---------- output projection ----------
        # wide 2-bank psum tiles per row-block, one eviction per block
        # alternating between DVE and ACT, DMA alternating rings
        with (
            tc.tile_pool(name="ypsum", bufs=2, space="PSUM") as ypsum,
            tc.tile_pool(name="ypool", bufs=3) as ypool,
        ):
            for nb in range(N // P):
                yp = ypsum.tile([P, 2, 512], F32, tag="yp")
                # hp-outer so the stationary operand changes once per
                # row-block instead of every matmul
                for hp in range(E // P):
                    for oc in range(DIM // 512):
                        nc.tensor.matmul(
                            yp[:, oc, :],
                            lhsT=otn2[:, hp, nb * P : (nb + 1) * P],
                            rhs=woT[:, hp, oc * 512 : (oc + 1) * 512],
                            start=(hp == 0),
                            stop=(hp == E // P - 1),
                        )
                ys = ypool.tile([P, 2, 512], BF16, tag="ys")
                if nb % 2:
                    nc.vector.tensor_copy(ys, yp)
                else:
                    nc.scalar.copy(ys, yp)
                ring = (nc.sync, nc.scalar, nc.gpsimd)[nb % 3]
                ring.dma_start(
                    out=y_d[nb * P : (nb + 1) * P, :], in_=ys
                )

    nc.compile()
    return nc


_NC_CACHE = []


def _get_nc():
    if not _NC_CACHE:
        _NC_CACHE.append(build_program())
    return _NC_CACHE[0]


def make_in_maps(x, context, mask, Wq, Wkv, b_kv, Wo):
    bf = ml_dtypes.bfloat16
    x = np.asarray(x, dtype=np.float32)
    context = np.asarray(context, dtype=np.float32)
    mask = np.asarray(mask)
    Wq = np.asarray(Wq, dtype=np.float32)
    Wkv = np.asarray(Wkv, dtype=np.float32)
    Wo = np.asarray(Wo, dtype=np.float32)

    def pmajor(aT):
        """[K, F] contraction-major -> [P, K//P, F] partition-major."""
        k, f = aT.shape
        return np.ascontiguousarray(
            aT.reshape(k // P, P, f).transpose(1, 0, 2)
        ).astype(bf)

    in_maps = []
    for b in range(B):
        xtb = pmajor(x[b].T)
        ctb = pmajor(context[b].T)
        mtb = pmajor(mask[b].T.astype(np.float32))
        for g in range(NUM_HEADS // HPC):
            sl = slice(E * g, E * (g + 1))
            in_maps.append(
                {
                    "xT": xtb,
                    "ctxT": ctb,
                    "maskt": mtb,
                    "wqT": pmajor(Wq[sl].T),
                    "wkT": pmajor(Wkv[sl].T),
                    "wvT": pmajor(Wkv[DIM + E * g : DIM + E * (g + 1)].T),
                    "woT": pmajor(Wo[:, sl].T),
                }
            )
    return in_maps


def combine_outputs(ys, b_kv, Wo):
    """ys: list of 8 per-core partial outputs [N, DIM], core order (b, g)."""
    b_v = np.asarray(b_kv, dtype=np.float32)[DIM:]
    ybias = np.asarray(Wo, dtype=np.float32) @ b_v  # [DIM]
    out = np.empty((B, N, DIM), dtype=np.float32)
    G = NUM_HEADS // HPC
    for b in range(B):
        acc = np.asarray(ys[G * b], dtype=np.float32)
        for g in range(1, G):
            acc = acc + np.asarray(ys[G * b + g], dtype=np.float32)
        out[b] = acc + ybias[None, :]
    return out


def kernel(x, context, mask, Wq, Wkv, b_kv, Wo):
    nc = _get_nc()
    in_maps = make_in_maps(x, context, mask, Wq, Wkv, b_kv, Wo)
    res = run_bass_kernel_spmd(nc, in_maps, core_ids=list(range(8)))
    ys = [m["y"] for m in res.results]
    return combine_outputs(ys, b_kv, Wo)


# revision 47
# speedup vs baseline: 1.0372x; 1.0372x over previous
"""CrossAttention Trainium2 kernel (8 NeuronCores, SPMD), bf16 compute.

Sharding: data-parallel over batch B=2, tensor-parallel over the 16 heads in
4 groups of 4 heads -> 8 cores, one (batch, head-group) pair each. Each core
computes its 4 heads' Q/K/V projections, masked softmax cross-attention, and
its partial output projection y_g = softmax(q k^T * scale) v @ Wo[:, g].T.
The host sums the 4 partial outputs per batch (the Wo row-split all-reduce,
done at unshard time) and adds the v-bias term Wo @ b_v, which is constant
across rows and factors out of the attention (softmax rows sum to 1).
The k-bias is dropped entirely: (k+b)^T q adds a per-query constant to the
logits, which softmax over keys is invariant to.

Numerics: inputs are cast to bf16 on the host; every matmul runs bf16 x bf16
with fp32 PSUM accumulation; softmax statistics (denominator, reciprocal,
normalization) stay fp32. Output partials are written bf16 and summed fp32
on the host. End-to-end relative error ~3e-3.

Layout: the PE contracts over the partition dim, so activations and weights
arrive contraction-major (pre-transposed on host); every device DMA is a
plain contiguous row load; no transposes on the device.

Attention is computed scores-transposed: ST[m, n] per head so PV contracts
over m directly; the two heads of a pair sit at partitions 0:64 / 64:128 so
their score matmuls run concurrently as PE row-tiles. The softmax
denominator comes free from a ones-column appended to v. exp() is
unnormalized (|s*scale| < ~4); mask zeros are applied multiplicatively
after exp with one wide (free=2048) DVE multiply per (pair, m-tile).

Stages (PE-dense, ACT overlapped):
  Q proj (chunk-major, DMA-paced) ->
  stage 1: scores+exp+mask heads 0,1 with K and V projections on PE slack
  stage 2: PV heads 0,1 interleaved with scores+exp+mask heads 2,3
  stage 3: PV head 2 (normalize heads 0,1 in its shadow), PV head 3
           (normalize head 2), normalize head 3, output projection.
Normalization: the denominator row is broadcast to 64 partitions with a
K=1 ones matmul on the PE (rhs read from the evicted ot_sb row at
partition 64), then DVE reciprocal + scale; odd heads shift into the upper
partition half via SBUF-SBUF DMA.
"""

import os

import numpy as np
import ml_dtypes

import concourse.bass as bass
import concourse.bacc as bacc
import concourse.mybir as mybir
import concourse.tile as tile
from concourse.bass_utils import run_bass_kernel_spmd

DIM = 1024
HEAD_DIM = 64
NUM_HEADS = 16
SCALE = HEAD_DIM**-0.5
B, N, M = 2, 1024, 2048
HPC = 4  # heads per core
E = HPC * HEAD_DIM  # 256: per-core projection width
P = 128
F32 = mybir.dt.float32
BF16 = mybir.dt.bfloat16
CT = DIM // P  # 8 contraction tiles
MT = M // P  # 16 m tiles


def build_program():
    nc = bacc.Bacc("TRN2", target_bir_lowering=False, debug=False, num_devices=8)

    # activation/weight shards arrive partition-major ([p, j, ...] with the
    # SBUF partition index outermost, pre-packed on the host) so every load
    # is one DMA with multi-KB contiguous runs per partition
    # x and ctx arrive as separate contiguous column-blocks: the completion
    # semaphore of a column-sliced (strided) DMA only fires at queue-drain
    # time, which would stall the first consumers by ~10us.
    xA_d = nc.dram_tensor("xA", [P, CT, 512], BF16, kind="ExternalInput").ap()
    xB_d = nc.dram_tensor("xB", [P, CT, 512], BF16, kind="ExternalInput").ap()
    ctx_d = [
        nc.dram_tensor(f"ctx{i}", [P, CT, 512], BF16, kind="ExternalInput").ap()
        for i in range(4)
    ]
    maskt_d = nc.dram_tensor("maskt", [P, MT, N], BF16, kind="ExternalInput").ap()
    wqT_d = nc.dram_tensor("wqT", [P, CT, E], BF16, kind="ExternalInput").ap()
    wkT_d = nc.dram_tensor("wkT", [P, CT, E], BF16, kind="ExternalInput").ap()
    wvT_d = nc.dram_tensor("wvT", [P, CT, E], BF16, kind="ExternalInput").ap()
    woT_d = nc.dram_tensor("woT", [P, E // P, DIM], BF16, kind="ExternalInput").ap()
    y_d = nc.dram_tensor("y", [N, DIM], BF16, kind="ExternalOutput").ap()

    kdbg = bool(os.environ.get("KDBG"))
    if kdbg:
        otdump_d = nc.dram_tensor(
            "otdump", [HEAD_DIM + 1, HPC, N], F32, kind="ExternalOutput"
        ).ap()

    Exp = mybir.ActivationFunctionType.Exp

    from contextlib import ExitStack

    with tile.TileContext(nc) as tc, ExitStack() as ctx:
        persist = ctx.enter_context(tc.tile_pool(name="persist", bufs=1))
        qT = persist.tile([P, E // P, N], BF16)
        kT = persist.tile([P, E // P, M], BF16)
        # v columns padded to 128 so the PV stationary is a full-width
        # weight load (enables Fast Weight Load; col 64 = ones for the
        # softmax denominator, cols 65:128 = don't-care)
        vaug = persist.tile([P, MT, HPC, P], BF16)
        woT = persist.tile([P, E // P, DIM], BF16)
        otn2 = persist.tile([P, E // P, N], BF16)
        ones_sb = persist.tile([P, HEAD_DIM], BF16)
        # rows 0:64 unnormalized attention out, row 64 denominator (bf16:
        # the ~0.4% rounding is well within the error budget and halves
        # the SBUF footprint; softmax statistics stay fp32 downstream)
        ot_sb = persist.tile([HEAD_DIM + 1, HPC, N], BF16)

        # ones column: fill everything; v evictions overwrite cols 0:64
        nc.vector.memset(vaug, 1.0)
        nc.vector.memset(ones_sb, 1.0)

        bwork = ctx.enter_context(tc.tile_pool(name="bwork", bufs=2))
        maskp = ctx.enter_context(tc.tile_pool(name="maskp", bufs=2))
        rbp = ctx.enter_context(tc.tile_pool(name="rbp", bufs=1))

        def emit_scores(spool, sbufs, hp, mt, exmst, mk):
            """scores -> exp for head pair hp at m-tile mt (per n-chunk PSUM
            tiles, double-buffered), then one wide masked multiply."""
            ex = bwork.tile([P, 2, N], BF16, tag="ex", name="ex")
            for chn in range(N // 512):
                st = spool.tile(
                    [P, 2, 512], F32, tag="st", name="st", bufs=sbufs
                )
                for hl in range(2):
                    erow = slice(hl * HEAD_DIM, (hl + 1) * HEAD_DIM)
                    nc.tensor.matmul(
                        st[:, hl, :],
                        lhsT=kT[erow, hp, mt * P : (mt + 1) * P],
                        rhs=qT[erow, hp, chn * 512 : (chn + 1) * 512],
                        start=True,
                        stop=True,
                    )
                nc.scalar.activation(
                    ex[:, :, chn * 512 : (chn + 1) * 512], st, Exp,
                    scale=float(SCALE),
                )
            mkc = bass.AP(mk.tensor, mk.offset, [mk.ap[0], [0, 2], mk.ap[1]])
            nc.vector.tensor_mul(exmst[:, mt, :, :], ex, mkc)

        def emit_pv(ot_ps, hp, mt, exmst):
            for hl in range(2):
                h = hp * 2 + hl
                for chn in range(N // 512):
                    nc.tensor.matmul(
                        ot_ps[hl * 2 + chn],
                        lhsT=vaug[:, mt, h, :],
                        rhs=exmst[:, mt, hl, chn * 512 : (chn + 1) * 512],
                        start=(mt == 0),
                        stop=(mt == MT - 1),
                    )

        def emit_pv1(ot_ps, h, mt, exmst):
            """PV for a single head h; ot_ps = [chn0, chn1] psum tiles."""
            hl = h % 2
            for chn in range(N // 512):
                nc.tensor.matmul(
                    ot_ps[chn],
                    lhsT=vaug[:, mt, h, :],
                    rhs=exmst[:, mt, hl, chn * 512 : (chn + 1) * 512],
                    start=(mt == 0),
                    stop=(mt == MT - 1),
                )

        def evict_head(ot_ps, h):
            """copy head h's two PV accumulators to ot_sb: denominator row
            first on the (idle) ACT engine so the normalize broadcast can
            start before the value rows land via DVE."""
            dn = slice(HEAD_DIM, HEAD_DIM + 1)
            for chn in range(2):
                nc.scalar.copy(
                    ot_sb[dn, h, chn * 512 : (chn + 1) * 512],
                    ot_ps[chn][dn, :],
                )
            for chn in range(2):
                nc.vector.tensor_copy(
                    ot_sb[:HEAD_DIM, h, chn * 512 : (chn + 1) * 512],
                    ot_ps[chn][:HEAD_DIM, :],
                )

        def normalize_head(h, rbq):
            """softmax-normalize head h from ot_sb into its otn2 half. The
            denominator row (partition 64) is broadcast to partitions 0:64
            with a K=1 bf16 ones matmul on the PE: rb_ps[d,n] = den[n]."""
            hp, hl = divmod(h, 2)
            dn = slice(HEAD_DIM, HEAD_DIM + 1)
            rb_ps = rbq.tile(
                [HEAD_DIM, N], F32, tag="rbps", name="rbps", bufs=1
            )
            for chn in range(2):
                nc.tensor.matmul(
                    rb_ps[:, chn * 512 : (chn + 1) * 512],
                    lhsT=ones_sb[HEAD_DIM : HEAD_DIM + 1, :],
                    rhs=ot_sb[dn, h, chn * 512 : (chn + 1) * 512],
                    start=True,
                    stop=True,
                )
            rb = rbp.tile([HEAD_DIM, N], F32, tag="rb", name="rb")
            nc.vector.reciprocal_approx_fast(out=rb, in_=rb_ps)
            if hl == 0:
                nc.vector.tensor_mul(
                    otn2[:HEAD_DIM, hp, :], ot_sb[:HEAD_DIM, h, :], rb
                )
            else:
                tmp = rbp.tile([HEAD_DIM, N], BF16, tag="tmp", name="tmp")
                nc.vector.tensor_mul(tmp, ot_sb[:HEAD_DIM, h, :], rb)
                # partition shift 0:64 -> 64:128 via SBUF-SBUF DMA
                nc.sync.dma_start(out=otn2[HEAD_DIM:P, hp, :], in_=tmp)

        def load_mask_group(g):
            """one 1MB DMA covering four m-tiles of the mask (big transfers
            spread over all 16 SDMA engines at ~3x the small-DMA rate)."""
            mkg = maskp.tile([P, 4, N], BF16, tag="mkg", name="mkg")
            nc.gpsimd.dma_start(out=mkg, in_=maskt_d[:, 4 * g : 4 * (g + 1), :])
            return mkg

        with tc.tile_pool(name="exmp", bufs=1) as exmp:
            # masked exp(scores) parked per m-tile; one buffer reused across
            # head pairs (WAR: stage-2 rewrites a tile only after its PV read)
            exmst = exmp.tile([P, MT, 2, N], BF16)

            with tc.tile_pool(name="wctx", bufs=1) as wctx_pool:
                wkT = wctx_pool.tile([P, CT, E], BF16)
                wvT = wctx_pool.tile([P, CT, E], BF16)
                # ctx as four column-block tiles (chm pieces)
                ctxT = [
                    wctx_pool.tile([P, CT, 512], BF16, name=f"ctxT{i}")
                    for i in range(4)
                ]

                with tc.tile_pool(name="qx", bufs=1) as qx_pool:
                    wqT = qx_pool.tile([P, CT, E], BF16)
                    xT = [
                        qx_pool.tile([P, CT, 512], BF16, name=f"xT{i}")
                        for i in range(2)
                    ]
                    # few, large DMAs: a single dma_start is split across all
                    # 16 SDMA engines and the partition-major host packing
                    # gives multi-KB contiguous runs (near-peak HBM rate).
                    # Dependency-first order; sync+scalar are the two HWDGE
                    # rings, gpsimd is the software ring.
                    nc.sync.dma_start(out=wqT, in_=wqT_d)
                    nc.sync.dma_start(out=xT[0], in_=xA_d)
                    nc.sync.dma_start(out=xT[1], in_=xB_d)
                    nc.scalar.dma_start(out=wkT, in_=wkT_d)
                    nc.scalar.dma_start(out=ctxT[0], in_=ctx_d[0])
                    nc.scalar.dma_start(out=ctxT[1], in_=ctx_d[1])
                    nc.sync.dma_start(out=ctxT[2], in_=ctx_d[2])
                    nc.sync.dma_start(out=ctxT[3], in_=ctx_d[3])
                    nc.gpsimd.dma_start(out=wvT, in_=wvT_d)
                    nc.gpsimd.dma_start(out=woT, in_=woT_d)

                    # Q projection, chunk-major so the first psum tile only
                    # needs wq + the first x half-tiles
                    with tc.tile_pool(name="ppsA", bufs=3, space="PSUM") as ppsA:
                        for chn in range(N // 512):
                            for et in range(E // P):
                                pq = ppsA.tile([P, 512], F32, tag="pq")
                                for j in range(CT):
                                    nc.tensor.matmul(
                                        pq,
                                        lhsT=wqT[:, j, et * P : (et + 1) * P],
                                        rhs=xT[chn][:, j, :],
                                        start=(j == 0),
                                        stop=(j == CT - 1),
                                    )
                                nc.vector.tensor_copy(
                                    qT[:, et, chn * 512 : (chn + 1) * 512], pq
                                )

                def emit_kproj(kps, et, chm):
                    pk = kps.tile([P, 512], F32, tag="pk", name="pk")
                    for j in range(CT):
                        nc.tensor.matmul(
                            pk,
                            lhsT=wkT[:, j, et * P : (et + 1) * P],
                            rhs=ctxT[chm][:, j, :],
                            start=(j == 0),
                            stop=(j == CT - 1),
                        )
                    nc.vector.tensor_copy(
                        kT[:, et, chm * 512 : (chm + 1) * 512], pk
                    )

                # K projection for the first head pair's first chunk must
                # precede stage 1; the rest is folded into stage 1's PE slack.
                # stage 1: scores(heads 0,1) [ACT-bound] + V and K
                # projections interleaved on the otherwise idle PE.
                with (
                    tc.tile_pool(name="sps1", bufs=1, space="PSUM") as sps1,
                    tc.tile_pool(name="vps", bufs=2, space="PSUM") as vps,
                    tc.tile_pool(name="kps", bufs=2, space="PSUM") as kps,
                ):
                    # kproj pacing: late enough that the tensor queue never
                    # blocks on not-yet-arrived ctx columns, early enough
                    # that kT stays ahead of the scores that consume it
                    # (et0 chunk c feeds scores mts 4c..4c+3; et1 feeds
                    # stage 2).
                    kproj_at = {4: (0, 1), 6: (0, 2), 8: (0, 3),
                                9: (1, 0), 11: (1, 1), 13: (1, 2), 15: (1, 3)}
                    emit_kproj(kps, 0, 0)
                    mkg = None
                    for mt in range(MT):
                        if et_chm := kproj_at.get(mt):
                            emit_kproj(kps, *et_chm)
                        if mt % 4 == 0:
                            mkg = load_mask_group(mt // 4)
                        emit_scores(sps1, 2, 0, mt, exmst, mkg[:, mt % 4, :])
                        pv = vps.tile([P, HPC, HEAD_DIM], F32, tag="pv")
                        mq, mr = divmod(mt, 4)
                        for j in range(CT):
                            nc.tensor.matmul(
                                pv,
                                lhsT=ctxT[mq][:, j, mr * P : (mr + 1) * P],
                                rhs=wvT[:, j, :],
                                start=(j == 0),
                                stop=(j == CT - 1),
                            )
                        # single fused eviction of all 4 heads' v columns
                        nc.vector.tensor_copy(
                            vaug[:, mt, :, :HEAD_DIM], pv
                        )

            # stage 2: PV(heads 0,1) interleaved with scores(heads 2,3)
            with tc.tile_pool(name="ops0", bufs=1, space="PSUM") as ops0:
                ot_ps0 = [
                    ops0.tile([P, 512], F32, tag=f"o{i}", name=f"o{i}")
                    for i in range(4)
                ]
                with tc.tile_pool(name="sps2", bufs=1, space="PSUM") as sps2:
                    mkg = None
                    for mt in range(MT):
                        if mt % 4 == 0:
                            mkg = load_mask_group(mt // 4)
                        emit_pv(ot_ps0, 0, mt, exmst)
                        emit_scores(sps2, 2, 1, mt, exmst, mkg[:, mt % 4, :])
                evict_head(ot_ps0[0:2], 0)
                evict_head(ot_ps0[2:4], 1)

            # stage 3: PV head 3 first (normalize heads 0,1 in its shadow),
            # then PV head 2; head 3's partition-shift DMA overlaps head 2's
            # PV so the last head (2, even -> no shift) has the shortest
            # possible normalize chain before the output projection.
            with (
                tc.tile_pool(name="ops1", bufs=1, space="PSUM") as ops1,
                tc.tile_pool(name="rbq", bufs=1, space="PSUM") as rbq,
            ):
                ot_ps3 = [
                    ops1.tile([P, 512], F32, tag=f"q{i}", name=f"q{i}")
                    for i in range(2)
                ]
                for mt in range(MT):
                    emit_pv1(ot_ps3, 3, mt, exmst)
                    if mt == 2:
                        normalize_head(0, rbq)
                    if mt == 8:
                        normalize_head(1, rbq)
                evict_head(ot_ps3, 3)
                ot_ps2 = [
                    ops1.tile([P, 512], F32, tag=f"p{i}", name=f"p{i}")
                    for i in range(2)
                ]
                for mt in range(MT):
                    emit_pv1(ot_ps2, 2, mt, exmst)
                    if mt == 2:
                        normalize_head(3, rbq)
                evict_head(ot_ps2, 2)
                normalize_head(2, rbq)

            if kdbg:
                nc.sync.dma_start(out=otdump_d, in_=ot_sb)

        # ---------- output projection ----------
        # wide 2-bank psum tiles per row-block, one eviction per block
        # alternating between DVE and ACT, DMA alternating rings
        with (
            tc.tile_pool(name="ypsum", bufs=3, space="PSUM") as ypsum,
            tc.tile_pool(name="ypool", bufs=3) as ypool,
        ):
            for nb in range(N // P):
                yp = ypsum.tile([P, 2, 512], F32, tag="yp")
                # hp-outer so the stationary operand changes once per
                # row-block instead of every matmul
                for hp in range(E // P):
                    for oc in range(DIM // 512):
                        nc.tensor.matmul(
                            yp[:, oc, :],
                            lhsT=otn2[:, hp, nb * P : (nb + 1) * P],
                            rhs=woT[:, hp, oc * 512 : (oc + 1) * 512],
                            start=(hp == 0),
                            stop=(hp == E // P - 1),
                        )
                ys = ypool.tile([P, 2, 512], BF16, tag="ys")
                if nb % 2:
                    nc.vector.tensor_copy(ys, yp)
                else:
                    nc.scalar.copy(ys, yp)
                ring = (nc.sync, nc.scalar, nc.gpsimd)[nb % 3]
                ring.dma_start(
                    out=y_d[nb * P : (nb + 1) * P, :], in_=ys
                )

    nc.compile()
    return nc


_NC_CACHE = []


def _get_nc():
    if not _NC_CACHE:
        _NC_CACHE.append(build_program())
    return _NC_CACHE[0]


def make_in_maps(x, context, mask, Wq, Wkv, b_kv, Wo):
    bf = ml_dtypes.bfloat16
    x = np.asarray(x, dtype=np.float32)
    context = np.asarray(context, dtype=np.float32)
    mask = np.asarray(mask)
    Wq = np.asarray(Wq, dtype=np.float32)
    Wkv = np.asarray(Wkv, dtype=np.float32)
    Wo = np.asarray(Wo, dtype=np.float32)

    def pmajor(aT):
        """[K, F] contraction-major -> [P, K//P, F] partition-major."""
        k, f = aT.shape
        return np.ascontiguousarray(
            aT.reshape(k // P, P, f).transpose(1, 0, 2)
        ).astype(bf)

    in_maps = []
    for b in range(B):
        xtb = pmajor(x[b].T)
        ctb = pmajor(context[b].T)
        mtb = pmajor(mask[b].T.astype(np.float32))
        xblocks = {
            "xA": np.ascontiguousarray(xtb[:, :, 0:512]),
            "xB": np.ascontiguousarray(xtb[:, :, 512:N]),
        }
        cblocks = {
            f"ctx{i}": np.ascontiguousarray(ctb[:, :, 512 * i : 512 * (i + 1)])
            for i in range(4)
        }
        for g in range(NUM_HEADS // HPC):
            sl = slice(E * g, E * (g + 1))
            in_maps.append(
                {
                    **xblocks,
                    **cblocks,
                    "maskt": mtb,
                    "wqT": pmajor(Wq[sl].T),
                    "wkT": pmajor(Wkv[sl].T),
                    "wvT": pmajor(Wkv[DIM + E * g : DIM + E * (g + 1)].T),
                    "woT": pmajor(Wo[:, sl].T),
                }
            )
    return in_maps


def combine_outputs(ys, b_kv, Wo):
    """ys: list of 8 per-core partial outputs [N, DIM], core order (b, g)."""
    b_v = np.asarray(b_kv, dtype=np.float32)[DIM:]
    ybias = np.asarray(Wo, dtype=np.float32) @ b_v  # [DIM]
    out = np.empty((B, N, DIM), dtype=np.float32)
    G = NUM_HEADS // HPC
    for b in range(B):
        acc = np.asarray(ys[G * b], dtype=np.float32)
        for g in range(1, G):
            acc = acc + np.asarray(ys[G * b + g], dtype=np.float32)
        out[b] = acc + ybias[None, :]
    return out


def kernel(x, context, mask, Wq, Wkv, b_kv, Wo):
    nc = _get_nc()
    in_maps = make_in_maps(x, context, mask, Wq, Wkv, b_kv, Wo)
    res = run_bass_kernel_spmd(nc, in_maps, core_ids=list(range(8)))
    ys = [m["y"] for m in res.results]
    return combine_outputs(ys, b_kv, Wo)
